# revision 18
# baseline (speedup 1.0000x reference)
"""GQA attention kernel for 8 TRN2 NeuronCores (Bass/Tile, SPMD).

Sharding: core c -> (batch b = c // 4, kv-head kv = c % 4). Each core computes
the 4 query heads of its kv group for its batch and a partial (transposed)
output projection; the host sums the 4 partials per batch.

v2: software-pipelined emission. The per-chunk projection chains (A) and the
per-chunk output projection (C) are interleaved as PE "filler" work between
attention (B) units, so the tensor engine never waits on the ACT-engine exp.
The softmax denominator is accumulated on DVE/Pool (split by k-tile parity)
into two SBUF tiles and reduced with two 1-row matmuls per (head, chunk),
replacing the per-tile ones-vector matmul (which cost as much PE time as PV).
DMA: x rides the sync HWDGE ring in 8 fine pieces per chunk (early WAR
release), weights ride the scalar HWDGE ring, y-stores ride the gpsimd SWDGE
ring, constants ride gpsimd too. otn/Wo are bf16 (error budget allows it).
"""

import os
import sys

import numpy as np

for _p in ("/opt/trn_rl_repo", "/root/.axon_site/_ro/trn_rl_repo"):
    if os.path.isdir(_p) and _p not in sys.path:
        sys.path.insert(0, _p)

import ml_dtypes  # noqa: E402

import concourse.bass as bass  # noqa: E402
import concourse.mybir as mybir  # noqa: E402
from concourse import bacc  # noqa: E402
from concourse.tile import TileContext  # noqa: E402
from concourse.bass_utils import run_bass_kernel_spmd  # noqa: E402

B, T, D = 2, 2048, 2048
H, HKV, HD = 16, 4, 128
G = H // HKV            # query heads per kv head (= per core)
EQ = G * HD             # 512: query-projection rows per core
P = 128
TC = 512                # t-chunk (free dim of most matmuls)
NJ = T // TC            # 4 chunks
DT = D // P             # 16 contraction tiles
NP = 8                  # x pieces per chunk (2 dt-tiles each)
SCALE = 1.0 / float(np.sqrt(HD))

F32 = mybir.dt.float32
F32R = mybir.dt.float32r
BF16 = mybir.dt.bfloat16
EXP = mybir.ActivationFunctionType.Exp

_CACHE = {}


def _build():
    nc = bacc.Bacc("TRN2", target_bir_lowering=False, debug=False)

    # All inputs arrive pre-transposed into SBUF layout (partition dim first,
    # contiguous per partition) so every DMA runs at full descriptor rate.
    xT = nc.declare_dram_parameter("xT", [P, NJ, NP, 2, TC], BF16, isOutput=False)
    wqT = nc.declare_dram_parameter("wqT", [P, DT, EQ], BF16, isOutput=False)
    wkT = nc.declare_dram_parameter("wkT", [P, DT, HD], BF16, isOutput=False)
    wvT = nc.declare_dram_parameter("wvT", [P, DT, HD], BF16, isOutput=False)
    woT = nc.declare_dram_parameter("woT", [P, G, D], BF16, isOutput=False)
    cosT = nc.declare_dram_parameter("cosT", [HD, T], BF16, isOutput=False)
    sinT = nc.declare_dram_parameter("sinT", [HD, T], BF16, isOutput=False)
    rmat = nc.declare_dram_parameter("rmat", [HD, HD], F32R, isOutput=False)
    iden = nc.declare_dram_parameter("iden", [P, P], BF16, isOutput=False)
    masks = nc.declare_dram_parameter("masks", [P, G, TC], BF16, isOutput=False)
    ones_k = nc.declare_dram_parameter("ones_k", [P, 1], BF16, isOutput=False)
    yT = nc.declare_dram_parameter("yT", [D, T], BF16, isOutput=True)

    with TileContext(nc) as tc:
        with (
            tc.tile_pool(name="const", bufs=1) as cst,
            tc.tile_pool(name="kv", bufs=1) as kvp,
            tc.tile_pool(name="ot", bufs=1) as otp,
            tc.tile_pool(name="wts", bufs=1) as wts,
            tc.tile_pool(name="xs", bufs=1) as xs,
            tc.tile_pool(name="qk", bufs=2) as qk,
            tc.tile_pool(name="work", bufs=5) as wk,
            tc.tile_pool(name="rtmp", bufs=2) as rtmp,
            tc.tile_pool(name="vt", bufs=2) as vtp,
            tc.tile_pool(name="small", bufs=2) as sml,
            tc.tile_pool(name="yev", bufs=4) as yev,
            tc.tile_pool(name="ps_acc", bufs=2, space="PSUM") as ps_acc,
            tc.tile_pool(name="ps_cp", bufs=1, space="PSUM") as ps_cp,
            tc.tile_pool(name="ps_s", bufs=2, space="PSUM") as ps_s,
            tc.tile_pool(name="ps_o", bufs=2, space="PSUM") as ps_o,
            tc.tile_pool(name="ps_lb", bufs=1, space="PSUM") as ps_lb,
        ):
            # ---- constants on the gpsimd SWDGE ring
            cos_sb = cst.tile([HD, T], BF16, tag="cos")
            sin_sb = cst.tile([HD, T], BF16, tag="sin")
            rmat_sb = cst.tile([HD, HD], F32R, tag="rmat")
            iden_sb = cst.tile([P, P], BF16, tag="iden")
            mask_sb = cst.tile([P, G, TC], BF16, tag="mask")
            onek_sb = cst.tile([P, 1], BF16, tag="onek")
            nc.gpsimd.dma_start(rmat_sb[:], rmat[:])
            nc.gpsimd.dma_start(iden_sb[:], iden[:])
            nc.gpsimd.dma_start(onek_sb[:], ones_k[:])

            # ---- weights on the scalar HWDGE ring
            wq_sb = wts.tile([P, DT, EQ], BF16, tag="wq")
            wk_sb = wts.tile([P, DT, HD], BF16, tag="wk")
            wv_sb = wts.tile([P, DT, HD], BF16, tag="wv")
            wo_sb = wts.tile([P, G, D], BF16, tag="wo")
            nc.scalar.dma_start(wv_sb[:], wvT[:])
            nc.scalar.dma_start(wk_sb[:], wkT[:])
            for q in range(4):
                nc.scalar.dma_start(wq_sb[:, 4 * q:4 * q + 4],
                                    wqT[:, 4 * q:4 * q + 4])
            nc.gpsimd.dma_start(cos_sb[:], cosT[:])
            nc.gpsimd.dma_start(sin_sb[:], sinT[:])
            nc.gpsimd.dma_start(mask_sb[:], masks[:])
            for g in range(G):
                nc.gpsimd.dma_start(wo_sb[:, g], woT[:, g])

            # per-chunk persistent tiles
            kts = [kvp.tile([HD, TC], F32R, tag=f"kt{j}", name=f"kt{j}") for j in range(NJ)]
            vch = [kvp.tile([P, 4, HD], BF16, tag=f"v{j}", name=f"v{j}") for j in range(NJ)]
            otn = [otp.tile([HD, G, TC], BF16, tag=f"o{j}", name=f"o{j}") for j in range(NJ)]
            qts = [None] * NJ

            def load_x(j):
                xp = [xs.tile([P, 2, TC], BF16, tag=f"xp{p}", name=f"xp{p}") for p in range(NP)]
                for p in range(NP):
                    nc.sync.dma_start(xp[p][:], xT[:, j, p])
                return xp

            xtiles = {0: load_x(0)}

            def rope_finish(s, t1, jsl):
                # s <- s*cos + rotate_half(s)*sin; t1 = s*cos precomputed
                pr = ps_s.tile([HD, TC], F32, tag="s", name="ropepr")
                nc.tensor.matmul(pr[:], rmat_sb[:], s, start=True, stop=True)
                nc.vector.tensor_mul(out=s, in0=pr[:], in1=sin_sb[:, jsl])
                nc.vector.tensor_add(out=s, in0=s, in1=t1[:])

            def a_stream(j, acc_pools=None):
                """Projection chains for chunk j; yields once per PE quantum."""
                pools = acc_pools or [(ps_acc, "acc")]
                jsl = slice(j * TC, (j + 1) * TC)
                xp = xtiles[j]
                qt = qk.tile([HD, G, TC], F32R, tag="qt", name="qt")
                qts[j] = qt
                prev_rope = None
                for a in range(6):
                    apool, atag = pools[a % len(pools)]
                    acc = apool.tile([P, TC], F32, tag=atag, name=f"acc{j}_{a}")
                    for dt in range(DT):
                        if a == 0:
                            lhsT = wv_sb[:, dt]
                        elif a == 1:
                            lhsT = wk_sb[:, dt]
                        else:
                            h = a - 2
                            lhsT = wq_sb[:, dt, h * HD:(h + 1) * HD]
                        nc.tensor.matmul(acc[:], lhsT, xp[dt // 2][:, dt % 2],
                                         start=(dt == 0), stop=(dt == DT - 1))
                        yield
                    if a == 0:
                        vt = vtp.tile([HD, TC], BF16, tag="vt", name="vt")
                        nc.scalar.copy(vt[:], acc[:])
                        for tt in range(4):
                            pvt = ps_s.tile([P, P], BF16, tag="s", name="pvt")
                            nc.tensor.transpose(pvt[:], vt[:, tt * P:(tt + 1) * P],
                                                iden_sb[:])
                            nc.vector.tensor_copy(vch[j][:, tt], pvt[:])
                            yield
                    else:
                        s = kts[j][:] if a == 1 else qt[:, a - 2]
                        nc.scalar.copy(s, acc[:])
                        t1 = rtmp.tile([HD, TC], F32R, tag="t1", name="t1")
                        nc.vector.tensor_mul(out=t1[:], in0=s, in1=cos_sb[:, jsl])
                        if prev_rope is not None:
                            rope_finish(*prev_rope)
                            yield
                        prev_rope = (s, t1, jsl)
                    if a == 5 and j + 1 < NJ:
                        # x for the next chunk: WAR on the xp tiles releases
                        # piece-by-piece as this chunk's Q3 chain retires.
                        xtiles[j + 1] = load_x(j + 1)
                        yield
                rope_finish(*prev_rope)
                yield

            def c_stream(j, cp_tag="cp", cp_pool=None):
                """Output projection for chunk j; yields once per matmul."""
                pool = cp_pool or ps_cp
                jsl = slice(j * TC, (j + 1) * TC)
                for dt in range(DT):
                    cp = pool.tile([P, TC], F32, tag=cp_tag, name=f"cp{j}_{dt}")
                    for g in range(G):
                        nc.tensor.matmul(cp[:], wo_sb[:, g, dt * P:(dt + 1) * P],
                                         otn[j][:, g], start=(g == 0),
                                         stop=(g == G - 1))
                        if g == G - 1:
                            ysb = yev.tile([P, TC], BF16, tag="ysb", name="ysb")
                            # alternate eviction engine; y rides the sync ring
                            if dt % 2:
                                nc.vector.tensor_copy(ysb[:], cp[:])
                            else:
                                nc.scalar.copy(ysb[:], cp[:])
                            nc.sync.dma_start(yT[dt * P:(dt + 1) * P, jsl],
                                              ysb[:])
                        yield

            # ---- A_0: plain emission
            for _ in a_stream(0):
                pass

            # ---- windows: B_j with A_{j+1} / C_{j-1} interleaved as fillers
            OFFS = {0: 0, 1: 128, 2: 256, 3: 256}
            DEPTH = 3

            for j in range(NJ):
                jsl = slice(j * TC, (j + 1) * TC)
                nk = 4 * (j + 1)
                fillers = []
                if j + 1 < NJ:
                    fillers.append(a_stream(j + 1))
                if j >= 1:
                    fillers.append(c_stream(j - 1))
                n_units = G * nk
                n_steps = (106 if j + 1 < NJ else 0) + (64 if j >= 1 else 0)
                R = max(1, -(-n_steps // n_units))

                rr = [0]

                def pump(n):
                    # round-robin across filler streams so A chains and C
                    # groups interleave (breaks same-ring WAR adjacency)
                    for _ in range(n):
                        while fillers:
                            rr[0] %= len(fillers)
                            try:
                                next(fillers[rr[0]])
                                rr[0] += 1
                                break
                            except StopIteration:
                                fillers.pop(rr[0])
                        if not fillers:
                            break

                qt = qts[j]
                po = {}
                pl = {}
                pipe = []

                def finalize(h):
                    rinv = sml.tile([1, TC], F32, tag="rinv", name="rinv")
                    nc.vector.reciprocal_approx_fast(rinv[:], pl[h][:])
                    binv = sml.tile([P, TC], F32, tag="binv", name="binv")
                    nc.gpsimd.partition_broadcast(binv[:], rinv[:])
                    nc.vector.tensor_mul(out=otn[j][:, h], in0=po[h][:],
                                         in1=binv[:])

                def drain():
                    ppt, ph, pkt, qs = pipe.pop(0)
                    m = pkt - 4 * j
                    pqs = slice(384, TC) if m == 3 else qs
                    nc.tensor.matmul(pl[ph][:, pqs], onek_sb[:], ppt[:, pqs],
                                     start=(pkt == 0), stop=(pkt == nk - 1))
                    nc.tensor.matmul(po[ph][:, pqs], vch[pkt // 4][:, pkt % 4],
                                     ppt[:, pqs], start=(pkt == 0),
                                     stop=(pkt == nk - 1))
                    if pkt == nk - 1:
                        finalize(ph)

                for h in range(G):
                    po[h] = ps_o.tile([P, TC], F32, tag="o", name=f"po{j}_{h}")
                    pl[h] = ps_lb.tile([1, TC], F32, tag="lb", name=f"pl{j}_{h}")
                    for kt in range(nk):
                        m = kt - 4 * j
                        off = 0 if m < 0 else OFFS[m]
                        qs = slice(off, TC)
                        pss = ps_s.tile([P, TC], F32, tag="s", name=f"ps{j}")
                        nc.tensor.matmul(pss[:, qs],
                                         kts[kt // 4][:, (kt % 4) * P:(kt % 4 + 1) * P],
                                         qt[:, h, qs], start=True, stop=True)
                        pt = wk.tile([P, TC], BF16, tag="pt", name="pt")
                        nc.scalar.activation(pt[:, qs], pss[:, qs], EXP,
                                             scale=SCALE)
                        if m >= 0:
                            ssl = slice(off, TC if m == 3 else off + P)
                            nc.vector.tensor_mul(out=pt[:, ssl], in0=pt[:, ssl],
                                                 in1=mask_sb[:, m, ssl])
                        pipe.append((pt, h, kt, qs))
                        if len(pipe) > DEPTH:
                            drain()
                        pump(R)
                while pipe:
                    drain()
                pump(1 << 30)

            # ---- C_3 tail on the freed ps_o ring (2 bufs avoid WAR stalls)
            for _ in c_stream(NJ - 1, cp_tag="o", cp_pool=ps_o):
                pass

    nc.compile()
    return nc


def _host_shards(inputs):
    x = np.ascontiguousarray(np.asarray(inputs["x"], dtype=np.float32))
    cos = np.asarray(inputs["cos"], dtype=np.float32)
    sin = np.asarray(inputs["sin"], dtype=np.float32)
    Wq = np.asarray(inputs["Wq"], dtype=np.float32)
    Wk = np.asarray(inputs["Wk"], dtype=np.float32)
    Wv = np.asarray(inputs["Wv"], dtype=np.float32)
    Wo = np.asarray(inputs["Wo"], dtype=np.float32)

    cosT = np.ascontiguousarray(cos.T).astype(ml_dtypes.bfloat16)
    sinT = np.ascontiguousarray(sin.T).astype(ml_dtypes.bfloat16)
    rmat = np.zeros((HD, HD), np.float32)
    half = HD // 2
    for i in range(half):
        rmat[i + half, i] = -1.0     # out[m<64] = -q[m+64]
        rmat[i, i + half] = 1.0      # out[m>=64] = q[m-64]
    iden = np.eye(P, dtype=ml_dtypes.bfloat16)
    kk = np.arange(P)[:, None, None]
    mm = np.arange(G)[None, :, None]
    qq = np.arange(TC)[None, None, :]
    masks = (qq >= kk + P * mm).astype(ml_dtypes.bfloat16)
    ones_k = np.ones((P, 1), ml_dtypes.bfloat16)

    def to_sbuf_layout(wT, cols):
        # [D_contract, cols] -> [P, D_contract//P, cols], partition dim first
        return np.ascontiguousarray(
            wT.reshape(-1, P, cols).transpose(1, 0, 2))

    # x[b].T is [d, t]; device layout [p, j, piece, s, t'] with
    # d = (2*piece+s)*P + p and t = j*TC + t' makes piece-loads contiguous.
    xTs = [np.ascontiguousarray(
        x[b].T.reshape(NP, 2, P, NJ, TC).transpose(2, 3, 0, 1, 4))
        .astype(ml_dtypes.bfloat16) for b in range(B)]
    bf = ml_dtypes.bfloat16
    wqTs = [to_sbuf_layout(Wq[kv * EQ:(kv + 1) * EQ].T, EQ).astype(bf)
            for kv in range(HKV)]
    wkTs = [to_sbuf_layout(Wk[kv * HD:(kv + 1) * HD].T, HD).astype(bf)
            for kv in range(HKV)]
    wvTs = [to_sbuf_layout(Wv[kv * HD:(kv + 1) * HD].T, HD).astype(bf)
            for kv in range(HKV)]
    woTs = [to_sbuf_layout(Wo[:, kv * EQ:(kv + 1) * EQ].T, D)
            .astype(ml_dtypes.bfloat16) for kv in range(HKV)]

    in_maps = []
    for c in range(8):
        b, kv = divmod(c, HKV)
        in_maps.append({
            "xT": xTs[b], "wqT": wqTs[kv], "wkT": wkTs[kv], "wvT": wvTs[kv],
            "woT": woTs[kv], "cosT": cosT, "sinT": sinT, "rmat": rmat,
            "iden": iden, "masks": masks, "ones_k": ones_k,
        })
    return in_maps


def get_nc():
    if "nc" not in _CACHE:
        _CACHE["nc"] = _build()
    return _CACHE["nc"]


def run(inputs, **kw):
    nc = get_nc()
    in_maps = _host_shards(inputs)
    res = run_bass_kernel_spmd(nc, in_maps, core_ids=list(range(8)), **kw)
    out = np.zeros((B, T, D), np.float32)
    for c in range(8):
        b = c // HKV
        out[b] += res.results[c]["yT"].astype(np.float32).T
    return out, res


def kernel(**inputs) -> np.ndarray:
    out, _ = run(inputs)
    return out


# revision 19
# speedup vs baseline: 1.0997x; 1.0997x over previous
"""GQA attention kernel for 8 TRN2 NeuronCores (Bass/Tile, SPMD).

Sharding: core c -> (batch b = c // 4, kv-head kv = c % 4). Each core computes
the 4 query heads of its kv group for its batch and a partial (transposed)
output projection; the host sums the 4 partials per batch.

v2: software-pipelined emission. The per-chunk projection chains (A) and the
per-chunk output projection (C) are interleaved as PE "filler" work between
attention (B) units, so the tensor engine never waits on the ACT-engine exp.
The softmax denominator is accumulated on DVE/Pool (split by k-tile parity)
into two SBUF tiles and reduced with two 1-row matmuls per (head, chunk),
replacing the per-tile ones-vector matmul (which cost as much PE time as PV).
DMA: x rides the sync HWDGE ring in 8 fine pieces per chunk (early WAR
release), weights ride the scalar HWDGE ring, y-stores ride the gpsimd SWDGE
ring, constants ride gpsimd too. otn/Wo are bf16 (error budget allows it).
"""

import os
import sys

import numpy as np

for _p in ("/opt/trn_rl_repo", "/root/.axon_site/_ro/trn_rl_repo"):
    if os.path.isdir(_p) and _p not in sys.path:
        sys.path.insert(0, _p)

import ml_dtypes  # noqa: E402

import concourse.bass as bass  # noqa: E402
import concourse.mybir as mybir  # noqa: E402
from concourse import bacc  # noqa: E402
from concourse.tile import TileContext  # noqa: E402
from concourse.bass_utils import run_bass_kernel_spmd  # noqa: E402

B, T, D = 2, 2048, 2048
H, HKV, HD = 16, 4, 128
G = H // HKV            # query heads per kv head (= per core)
EQ = G * HD             # 512: query-projection rows per core
P = 128
TC = 512                # t-chunk (free dim of most matmuls)
NJ = T // TC            # 4 chunks
DT = D // P             # 16 contraction tiles
NP = 8                  # x pieces per chunk (2 dt-tiles each)
SCALE = 1.0 / float(np.sqrt(HD))

F32 = mybir.dt.float32
F32R = mybir.dt.float32r
BF16 = mybir.dt.bfloat16
EXP = mybir.ActivationFunctionType.Exp

_CACHE = {}


def _build():
    nc = bacc.Bacc("TRN2", target_bir_lowering=False, debug=False)

    # All inputs arrive pre-transposed into SBUF layout (partition dim first,
    # contiguous per partition) so every DMA runs at full descriptor rate.
    xT = nc.declare_dram_parameter("xT", [P, NJ, NP, 2, TC], BF16, isOutput=False)
    wqT = nc.declare_dram_parameter("wqT", [P, DT, EQ], BF16, isOutput=False)
    wkT = nc.declare_dram_parameter("wkT", [P, DT, HD], BF16, isOutput=False)
    wvT = nc.declare_dram_parameter("wvT", [P, DT, HD], BF16, isOutput=False)
    woT = nc.declare_dram_parameter("woT", [P, G, D], BF16, isOutput=False)
    cosT = nc.declare_dram_parameter("cosT", [HD, T], BF16, isOutput=False)
    sinT = nc.declare_dram_parameter("sinT", [HD, T], BF16, isOutput=False)
    rmat = nc.declare_dram_parameter("rmat", [HD, HD], F32R, isOutput=False)
    iden = nc.declare_dram_parameter("iden", [P, P], F32R, isOutput=False)
    masks = nc.declare_dram_parameter("masks", [P, G, TC], F32R, isOutput=False)
    ones_k = nc.declare_dram_parameter("ones_k", [P, 1], F32R, isOutput=False)
    yT = nc.declare_dram_parameter("yT", [D, T], BF16, isOutput=True)

    with TileContext(nc) as tc:
        with (
            tc.tile_pool(name="const", bufs=1) as cst,
            tc.tile_pool(name="kv", bufs=1) as kvp,
            tc.tile_pool(name="ot", bufs=1) as otp,
            tc.tile_pool(name="wts", bufs=1) as wts,
            tc.tile_pool(name="xs", bufs=1) as xs,
            tc.tile_pool(name="qk", bufs=2) as qk,
            tc.tile_pool(name="work", bufs=5) as wk,
            tc.tile_pool(name="rtmp", bufs=2) as rtmp,
            tc.tile_pool(name="vt", bufs=2) as vtp,
            tc.tile_pool(name="small", bufs=2) as sml,
            tc.tile_pool(name="yev", bufs=4) as yev,
            tc.tile_pool(name="ps_acc", bufs=2, space="PSUM") as ps_acc,
            tc.tile_pool(name="ps_cp", bufs=1, space="PSUM") as ps_cp,
            tc.tile_pool(name="ps_s", bufs=2, space="PSUM") as ps_s,
            tc.tile_pool(name="ps_o", bufs=2, space="PSUM") as ps_o,
            tc.tile_pool(name="ps_lb", bufs=1, space="PSUM") as ps_lb,
        ):
            # ---- constants on the gpsimd SWDGE ring
            cos_sb = cst.tile([HD, T], BF16, tag="cos")
            sin_sb = cst.tile([HD, T], BF16, tag="sin")
            rmat_sb = cst.tile([HD, HD], F32R, tag="rmat")
            iden_sb = cst.tile([P, P], F32R, tag="iden")
            mask_sb = cst.tile([P, G, TC], F32R, tag="mask")
            onek_sb = cst.tile([P, 1], F32R, tag="onek")
            nc.gpsimd.dma_start(rmat_sb[:], rmat[:])
            nc.gpsimd.dma_start(iden_sb[:], iden[:])
            nc.gpsimd.dma_start(onek_sb[:], ones_k[:])

            # ---- weights on the scalar HWDGE ring
            wq_sb = wts.tile([P, DT, EQ], BF16, tag="wq")
            wk_sb = wts.tile([P, DT, HD], BF16, tag="wk")
            wv_sb = wts.tile([P, DT, HD], BF16, tag="wv")
            wo_sb = wts.tile([P, G, D], BF16, tag="wo")
            nc.scalar.dma_start(wv_sb[:], wvT[:])
            for q in range(2):
                nc.scalar.dma_start(wq_sb[:, 4 * q:4 * q + 4],
                                    wqT[:, 4 * q:4 * q + 4])
            nc.gpsimd.dma_start(wk_sb[:], wkT[:])
            nc.gpsimd.dma_start(cos_sb[:], cosT[:])
            nc.gpsimd.dma_start(sin_sb[:], sinT[:])
            for q in range(2, 4):
                nc.gpsimd.dma_start(wq_sb[:, 4 * q:4 * q + 4],
                                    wqT[:, 4 * q:4 * q + 4])
            nc.gpsimd.dma_start(mask_sb[:], masks[:])
            for g in range(G):
                nc.gpsimd.dma_start(wo_sb[:, g], woT[:, g])

            # per-chunk persistent tiles
            kts = [kvp.tile([HD, TC], F32R, tag=f"kt{j}", name=f"kt{j}") for j in range(NJ)]
            vch = [kvp.tile([P, 4, HD], F32R, tag=f"v{j}", name=f"v{j}") for j in range(NJ)]
            otn = [otp.tile([HD, G, TC], BF16, tag=f"o{j}", name=f"o{j}") for j in range(NJ)]
            qts = [None] * NJ

            def load_x(j):
                xp = [xs.tile([P, 2, TC], BF16, tag=f"xp{p}", name=f"xp{p}") for p in range(NP)]
                for p in range(NP):
                    nc.sync.dma_start(xp[p][:], xT[:, j, p])
                return xp

            xtiles = {0: load_x(0)}

            def rope_finish(s, t1, jsl):
                # s <- s*cos + rotate_half(s)*sin; t1 = s*cos precomputed
                pr = ps_s.tile([HD, TC], F32, tag="s", name="ropepr")
                nc.tensor.matmul(pr[:], rmat_sb[:], s, start=True, stop=True)
                nc.vector.tensor_mul(out=s, in0=pr[:], in1=sin_sb[:, jsl])
                nc.vector.tensor_add(out=s, in0=s, in1=t1[:])

            def a_stream(j, acc_pools=None):
                """Projection chains for chunk j; yields once per PE quantum."""
                pools = acc_pools or [(ps_acc, "acc")]
                jsl = slice(j * TC, (j + 1) * TC)
                xp = xtiles[j]
                qt = qk.tile([HD, G, TC], F32R, tag="qt", name="qt")
                qts[j] = qt
                prev_rope = None
                for a in range(6):
                    apool, atag = pools[a % len(pools)]
                    acc = apool.tile([P, TC], F32, tag=atag, name=f"acc{j}_{a}")
                    for dt in range(DT):
                        if a == 0:
                            lhsT = wv_sb[:, dt]
                        elif a == 1:
                            lhsT = wk_sb[:, dt]
                        else:
                            h = a - 2
                            lhsT = wq_sb[:, dt, h * HD:(h + 1) * HD]
                        nc.tensor.matmul(acc[:], lhsT, xp[dt // 2][:, dt % 2],
                                         start=(dt == 0), stop=(dt == DT - 1))
                        yield
                    if a == 0:
                        vt = vtp.tile([HD, TC], F32R, tag="vt", name="vt")
                        nc.scalar.copy(vt[:], acc[:])
                        for tt in range(4):
                            pvt = ps_s.tile([P, P], F32R, tag="s", name="pvt")
                            nc.tensor.transpose(pvt[:], vt[:, tt * P:(tt + 1) * P],
                                                iden_sb[:])
                            nc.vector.tensor_copy(vch[j][:, tt], pvt[:])
                            yield
                    else:
                        s = kts[j][:] if a == 1 else qt[:, a - 2]
                        nc.scalar.copy(s, acc[:])
                        t1 = rtmp.tile([HD, TC], F32R, tag="t1", name="t1")
                        nc.vector.tensor_mul(out=t1[:], in0=s, in1=cos_sb[:, jsl])
                        if prev_rope is not None:
                            rope_finish(*prev_rope)
                            yield
                        prev_rope = (s, t1, jsl)
                    if a == 5 and j + 1 < NJ:
                        # x for the next chunk: WAR on the xp tiles releases
                        # piece-by-piece as this chunk's Q3 chain retires.
                        xtiles[j + 1] = load_x(j + 1)
                        yield
                rope_finish(*prev_rope)
                yield

            def c_stream(j, cp_tag="cp", cp_pool=None):
                """Output projection for chunk j; yields once per matmul."""
                pool = cp_pool or ps_cp
                jsl = slice(j * TC, (j + 1) * TC)
                for dt in range(DT):
                    cp = pool.tile([P, TC], F32, tag=cp_tag, name=f"cp{j}_{dt}")
                    for g in range(G):
                        nc.tensor.matmul(cp[:], wo_sb[:, g, dt * P:(dt + 1) * P],
                                         otn[j][:, g], start=(g == 0),
                                         stop=(g == G - 1))
                        if g == G - 1:
                            ysb = yev.tile([P, TC], BF16, tag="ysb", name="ysb")
                            # alternate eviction engine; y rides the sync ring
                            if dt % 2:
                                nc.vector.tensor_copy(ysb[:], cp[:])
                            else:
                                nc.scalar.copy(ysb[:], cp[:])
                            nc.sync.dma_start(yT[dt * P:(dt + 1) * P, jsl],
                                              ysb[:])
                        yield

            # ---- A_0: plain emission
            for _ in a_stream(0):
                pass

            # ---- windows: B_j with A_{j+1} / C_{j-1} interleaved as fillers
            OFFS = {0: 0, 1: 128, 2: 256, 3: 256}
            DEPTH = 3

            for j in range(NJ):
                jsl = slice(j * TC, (j + 1) * TC)
                nk = 4 * (j + 1)
                fillers = []
                if j + 1 < NJ:
                    fillers.append(a_stream(j + 1))
                if j >= 1:
                    fillers.append(c_stream(j - 1))
                n_units = G * nk
                n_steps = (106 if j + 1 < NJ else 0) + (64 if j >= 1 else 0)
                R = max(1, -(-n_steps // n_units))

                rr = [0]

                def pump(n):
                    # round-robin across filler streams so A chains and C
                    # groups interleave (breaks same-ring WAR adjacency)
                    for _ in range(n):
                        while fillers:
                            rr[0] %= len(fillers)
                            try:
                                next(fillers[rr[0]])
                                rr[0] += 1
                                break
                            except StopIteration:
                                fillers.pop(rr[0])
                        if not fillers:
                            break

                qt = qts[j]
                po = {}
                pl = {}
                pipe = []

                def finalize(h):
                    rinv = sml.tile([1, TC], F32, tag="rinv", name="rinv")
                    nc.vector.reciprocal_approx_fast(rinv[:], pl[h][:])
                    binv = sml.tile([P, TC], F32, tag="binv", name="binv")
                    nc.gpsimd.partition_broadcast(binv[:], rinv[:])
                    nc.vector.tensor_mul(out=otn[j][:, h], in0=po[h][:],
                                         in1=binv[:])

                def drain():
                    ppt, ph, pkt, qs = pipe.pop(0)
                    pqs = qs
                    nc.tensor.matmul(pl[ph][:, pqs], onek_sb[:], ppt[:, pqs],
                                     start=(pkt == 0), stop=(pkt == nk - 1))
                    nc.tensor.matmul(po[ph][:, pqs], vch[pkt // 4][:, pkt % 4],
                                     ppt[:, pqs], start=(pkt == 0),
                                     stop=(pkt == nk - 1))
                    if pkt == nk - 1:
                        finalize(ph)

                for h in range(G):
                    po[h] = ps_o.tile([P, TC], F32, tag="o", name=f"po{j}_{h}")
                    pl[h] = ps_lb.tile([1, TC], F32, tag="lb", name=f"pl{j}_{h}")
                    for kt in range(nk):
                        m = kt - 4 * j
                        off = 0 if m < 0 else OFFS[m]
                        qs = slice(off, TC)
                        pss = ps_s.tile([P, TC], F32, tag="s", name=f"ps{j}")
                        nc.tensor.matmul(pss[:, qs],
                                         kts[kt // 4][:, (kt % 4) * P:(kt % 4 + 1) * P],
                                         qt[:, h, qs], start=True, stop=True)
                        pt = wk.tile([P, TC], F32R, tag="pt", name="pt")
                        nc.scalar.activation(pt[:, qs], pss[:, qs], EXP,
                                             scale=SCALE)
                        if m >= 0:
                            ssl = slice(off, TC if m == 3 else off + P)
                            nc.vector.tensor_mul(out=pt[:, ssl], in0=pt[:, ssl],
                                                 in1=mask_sb[:, m, ssl])
                        pipe.append((pt, h, kt, qs))
                        if len(pipe) > DEPTH:
                            drain()
                        pump(R)
                while pipe:
                    drain()
                pump(1 << 30)

            # ---- C_3 tail on the freed ps_o ring (2 bufs avoid WAR stalls)
            for _ in c_stream(NJ - 1, cp_tag="o", cp_pool=ps_o):
                pass

    nc.compile()
    return nc


def _host_shards(inputs):
    x = np.ascontiguousarray(np.asarray(inputs["x"], dtype=np.float32))
    cos = np.asarray(inputs["cos"], dtype=np.float32)
    sin = np.asarray(inputs["sin"], dtype=np.float32)
    Wq = np.asarray(inputs["Wq"], dtype=np.float32)
    Wk = np.asarray(inputs["Wk"], dtype=np.float32)
    Wv = np.asarray(inputs["Wv"], dtype=np.float32)
    Wo = np.asarray(inputs["Wo"], dtype=np.float32)

    cosT = np.ascontiguousarray(cos.T).astype(ml_dtypes.bfloat16)
    sinT = np.ascontiguousarray(sin.T).astype(ml_dtypes.bfloat16)
    rmat = np.zeros((HD, HD), np.float32)
    half = HD // 2
    for i in range(half):
        rmat[i + half, i] = -1.0     # out[m<64] = -q[m+64]
        rmat[i, i + half] = 1.0      # out[m>=64] = q[m-64]
    iden = np.eye(P, dtype=np.float32)
    kk = np.arange(P)[:, None, None]
    mm = np.arange(G)[None, :, None]
    qq = np.arange(TC)[None, None, :]
    masks = (qq >= kk + P * mm).astype(np.float32)
    ones_k = np.ones((P, 1), np.float32)

    def to_sbuf_layout(wT, cols):
        # [D_contract, cols] -> [P, D_contract//P, cols], partition dim first
        return np.ascontiguousarray(
            wT.reshape(-1, P, cols).transpose(1, 0, 2))

    # x[b].T is [d, t]; device layout [p, j, piece, s, t'] with
    # d = (2*piece+s)*P + p and t = j*TC + t' makes piece-loads contiguous.
    xTs = [np.ascontiguousarray(
        x[b].T.reshape(NP, 2, P, NJ, TC).transpose(2, 3, 0, 1, 4))
        .astype(ml_dtypes.bfloat16) for b in range(B)]
    bf = ml_dtypes.bfloat16
    wqTs = [to_sbuf_layout(Wq[kv * EQ:(kv + 1) * EQ].T, EQ).astype(bf)
            for kv in range(HKV)]
    wkTs = [to_sbuf_layout(Wk[kv * HD:(kv + 1) * HD].T, HD).astype(bf)
            for kv in range(HKV)]
    wvTs = [to_sbuf_layout(Wv[kv * HD:(kv + 1) * HD].T, HD).astype(bf)
            for kv in range(HKV)]
    woTs = [to_sbuf_layout(Wo[:, kv * EQ:(kv + 1) * EQ].T, D)
            .astype(ml_dtypes.bfloat16) for kv in range(HKV)]

    in_maps = []
    for c in range(8):
        b, kv = divmod(c, HKV)
        in_maps.append({
            "xT": xTs[b], "wqT": wqTs[kv], "wkT": wkTs[kv], "wvT": wvTs[kv],
            "woT": woTs[kv], "cosT": cosT, "sinT": sinT, "rmat": rmat,
            "iden": iden, "masks": masks, "ones_k": ones_k,
        })
    return in_maps


def get_nc():
    if "nc" not in _CACHE:
        _CACHE["nc"] = _build()
    return _CACHE["nc"]


def run(inputs, **kw):
    nc = get_nc()
    in_maps = _host_shards(inputs)
    res = run_bass_kernel_spmd(nc, in_maps, core_ids=list(range(8)), **kw)
    out = np.zeros((B, T, D), np.float32)
    for c in range(8):
        b = c // HKV
        out[b] += res.results[c]["yT"].astype(np.float32).T
    return out, res


def kernel(**inputs) -> np.ndarray:
    out, _ = run(inputs)
    return out
